# revision 70
# baseline (speedup 1.0000x reference)
"""Trainium2 Bass kernel for nn_CausalVAE (encoder MLP + reparam + 32-node
sequential causal decoder).

Sharding: data-parallel over batch across 8 NeuronCores (4096 rows/core),
weights replicated. On-chip layout is feature-major (features on SBUF
partitions, batch on the free dim) so every layer is a chain of
[K<=128, M<=128] x [K, 512] fp32r matmuls with no on-chip transposes
(inputs/outputs are transposed on the host as part of shard/gather).
The per-node causal masking is exact: node i's first matmul reads only
the first i+1 partitions of the running y^T state tile.

Key device-side structure:
- b3 is folded out of the per-node epilogue on the HOST (decoded rows live
  without b3; the deficit is a per-node constant folded into the relu1
  bias b1), so the per-node epilogue is one copy + one row DMA;
- the encoder streams out et=exp(lv/2) (fused psum->exp activation) and
  y0 (one fused scalar_tensor_tensor) instead of mu/lv; the host
  reconstructs lv = 2*ln(et) and mu = y0 - et*eps exactly;
- all DMAs share one FIFO (SP queue -> descriptor gen -> DMA engines) at a
  flat ~1.3us per descriptor, so ordering AND count are everything: the
  small weights are host-packed into three bundled tensors (bep/Wep/bwp)
  in their on-chip layouts, W2/W1 tails stream from inside the decoder
  loop by first-use time, etT streams out in the mid-decode lull, and the
  y0 output DMAs ride the FIFO ahead of the row DMAs that would overwrite
  their source rows;
- node 31 rows bypass the yT state and stream straight to the output, and
  per-tile output DMAs for rows 0..30 are emitted inside the pipeline loop
  so the tail is just the last row's DMA;
- throwaway warm-up matmuls ramp the PE p-state and prefetch the
  activation table while the first input DMAs are in flight;
- the decoder is a 4-deep software pipeline (engine queues execute in
  emission order), with its first steps interleaved into the encoder loop
  to fill PE slots while the encoder's DVE/Act reparam chain drains.

Matmul precision switchable via VAE_MM_MODE: float32r (default, tf32-like,
full speed) or float32 (exact, 4x slower).
"""

import os

import numpy as np

import concourse.bass as bass
import concourse.mybir as mybir
import concourse.tile as tile
from concourse import bacc
from concourse.alu_op_type import AluOpType
from concourse.bass import ts
from concourse.bass_utils import run_bass_kernel_spmd

D = 32          # causal nodes / feature dim of y
DF = 256        # hidden dim
B = 32768       # full batch
NCORES = 8
BL = B // NCORES          # 4096 rows per core
BT = 512                  # batch tile (matmul moving free dim)
NBT = BL // BT            # 8 batch tiles per core

F32 = mybir.dt.float32
F32R = mybir.dt.float32r
AF = mybir.ActivationFunctionType

# "float32r" (fast, tf32-like) or "float32" (exact, 4x slower matmul)
MM_MODE = os.environ.get("VAE_MM_MODE", "float32r")
# decoded rows: DMA straight from PSUM (1) or stage through SBUF (0)
SROW_DMA = os.environ.get("VAE_SROW_DMA", "0") == "1"
# srow copy engine split "M:T": Act when k %% M < T, DVE otherwise
_SM, _ST = os.environ.get("VAE_SROW", "2:1").split(":")
SROW_MOD, SROW_THR = int(_SM), int(_ST)
# how many decoder steps interleave into the encoder loop tail
MERGE0 = int(os.environ.get("VAE_MERGE0", "3"))
# decoder steps k < KPOOL run relu1-a and the srow copy on the (otherwise
# idle) Pool engine: both have multi-step consumer slack, and Act/DVE are
# the pace-setters while the encoder drains
KPOOL = int(os.environ.get("VAE_KPOOL", "0"))


def _make_nc():
    rmode = MM_MODE == "float32r"

    def r(ap):
        """View an AP as float32r (matmul operands + their producers)."""
        return ap.bitcast(F32R) if rmode else ap

    nc = bacc.Bacc("TRN2", target_bir_lowering=False, debug=False)

    # ---- DRAM I/O (activations pre-transposed on host: [feat, batch]) ----
    # b1p/b2p/W3p/be1p/be2p are pre-packed on the host into the on-chip
    # layouts (b1p also carries the b3 fold, see _run).
    xorT_d = nc.dram_tensor("xorT", [D, BL], F32, kind="ExternalInput")
    epsT_d = nc.dram_tensor("epsT", [D, BL], F32, kind="ExternalInput")
    We1_d = nc.dram_tensor("We1", [D, DF], F32, kind="ExternalInput")
    # host-packed bundles: the startup FIFO costs ~0.65-1.3us PER DMA
    # (flat descriptor cost), so the small constants ship as single DMAs.
    # bep = [be1a be1b be2a be2b] columns; Wep = [We2 | We3] along the free
    # dim; bwp = [b1p | b2p | W3p | be3pad] along the free dim.
    bep_d = nc.dram_tensor("bep", [128, 4], F32, kind="ExternalInput")
    Wep_d = nc.dram_tensor("Wep", [128, 2, DF + 2 * D], F32,
                           kind="ExternalInput")
    W1_d = nc.dram_tensor("W1", [D, D, DF], F32, kind="ExternalInput")
    W2_d = nc.dram_tensor("W2", [D, DF, DF], F32, kind="ExternalInput")
    bwp_d = nc.dram_tensor("bwp", [128, 6 * D + 1], F32,
                           kind="ExternalInput")
    yT_dr = nc.dram_tensor("yT", [D, BL], F32, kind="ExternalOutput")
    # the encoder streams out et=exp(lv/2) and y0 instead of mu/lv; the
    # host reconstructs lv = 2*ln(et), mu = y0 - et*eps exactly. This keeps
    # the whole reparam staging to one Act op + two DVE ops per tile.
    y0T_d = nc.dram_tensor("y0T", [D, BL], F32, kind="ExternalOutput")
    etT_d = nc.dram_tensor("etT", [D, BL], F32, kind="ExternalOutput")

    # PZPAIR: two tiles' pz share one [64, 2*BT] psum tile (same partitions,
    # adjacent free ranges) so the reparam tail (et/exp, tm, y0) runs one
    # 1024-wide op per PAIR instead of two 512-wide ops -- the encoder's
    # serial Act/DVE chains are the window pace-setters. Costs 2 psum banks,
    # paid for by psum=5/psum3=1.
    PZPAIR = os.environ.get("VAE_PZPAIR", "0") == "1"
    with tile.TileContext(nc) as tc:
        with (
            tc.tile_pool(name="wpool", bufs=1) as wp,
            tc.tile_pool(name="actp", bufs=1) as actp,
            tc.tile_pool(name="psum", bufs=4 if PZPAIR else 6,
                         space="PSUM") as psp,
            tc.tile_pool(name="psum3", bufs=2, space="PSUM") as ps3,
            tc.tile_pool(name="psumz", bufs=1, space="PSUM") as psz,
        ):
            # ---- persistent feature-major activations ----
            xorT = actp.tile([D, BL], F32)       # xor^T
            epsT = actp.tile([D, BL], F32)       # eps^T
            etT = actp.tile([D, BL], F32)        # exp(lv/2)^T
            # ONE yT state tile [D, BL]: decoded rows are engine-written
            # (psum->sbuf copy) straight into it, so the state path never
            # touches the serial DMA FIFO (was 248 row DMAs at ~1.3us of
            # SP+HWDGE each)
            yT = actp.tile([D, BL], F32, name="yT")

            # ---- PE warm-up ----
            # The tensor engine p-state ramps to full clock only after ~3us
            # of continuous execution. Run throwaway matmuls on a zeroed
            # scratch tile from t~0 so the ramp completes while the first
            # input DMAs are still in flight.
            wmsc = wp.tile([1, 256], F32)
            wmset = os.environ.get("VAE_WMSET", "pool")
            if wmset == "act":
                # Act memzero starts ~30ns in and finishes ~450ns before the
                # Pool Q7 launch path wakes up (r(): the consumer matmuls
                # read wmsc as f32r, and the BIR verifier requires engine
                # writes into f32r-consumed tiles to be rounded)
                nc.scalar.memzero(r(wmsc[:]))
            else:
                # Pool-engine memset: starts ~60ns in (DVE's first op pays
                # ~700ns of decode/queue latency; Pool memsets are proven in
                # the framework preamble)
                nc.gpsimd.memset(wmsc[:], 0.0)
            # dummy activation: hoists the one-time 1.3us activation-table
            # load off the encoder critical path, into the DMA wait
            wmact = wp.tile([1, 256], F32)
            nc.scalar.activation(wmact[:], wmsc[:], AF.Relu)
            for _ in range(int(os.environ.get("VAE_WARMN", "8"))):
                wmps = ps3.tile([128, BT], F32, tag="p3", name="wmps")
                nc.tensor.matmul(wmps[:, 0:256], r(wmsc[:, 0:128]), r(wmsc[:]),
                                 start=True, stop=True)

            # ---- DMAs ordered by first use (startup-critical first) ----
            We1sb = wp.tile([D, DF], F32)
            nc.sync.dma_start(out=r(We1sb[:]), in_=r(We1_d[:]))
            if os.environ.get("VAE_XSPLIT", "0") == "1":
                # first batch tile alone so enc_mm(0) can start ~1.2us sooner
                nc.sync.dma_start(out=r(xorT[:, 0:BT]),
                                  in_=r(xorT_d[:, 0:BT]))
                nc.sync.dma_start(out=r(xorT[:, BT:BL // 4]),
                                  in_=r(xorT_d[:, BT:BL // 4]))
            else:
                nc.sync.dma_start(out=r(xorT[:, 0:BL // 4]),
                                  in_=r(xorT_d[:, 0:BL // 4]))
            # one [128,4] DMA: be1/be2 as adjacent column pairs
            bepsb = wp.tile([128, 4], F32)
            nc.sync.dma_start(out=bepsb[:], in_=bep_d[:])
            be1sb = bepsb[:, 0:2]
            be2sb = bepsb[:, 2:4]
            # one DMA for We2|We3 (packed along the free dim on the host)
            Wepsb = wp.tile([128, 2, DF + 2 * D], F32)
            nc.sync.dma_start(out=r(Wepsb[:]), in_=r(Wep_d[:]))
            We2sb = Wepsb[:, :, 0:DF]
            We3sb = Wepsb[:, :, DF:DF + 2 * D]
            # one DMA for b1p|b2p|W3p|be3: bwp = [128, 6D+1]. be3x is
            # [be3_mu ; be3_lv/2] host-packed (the lv half pre-halved so
            # et = Exp(pz*0.5 + be3x) computes exp((pz+be3_lv)/2) straight
            # from psum), padded to 128 partitions in the last column
            bwpsb = wp.tile([128, 6 * D + 1], F32)
            nc.sync.dma_start(out=r(bwpsb[:]), in_=r(bwp_d[:]))
            b1sb = bwpsb[:, 0:2 * D].rearrange("p (i j) -> p i j", j=2)
            b2sb = bwpsb[:, 2 * D:4 * D].rearrange("p (i j) -> p i j", j=2)
            W3sb = bwpsb[:, 4 * D:6 * D].rearrange("p (i j) -> p i j", j=2)
            be3sb = bwpsb[0:2 * D, 6 * D:6 * D + 1]
            # interleave the remaining input chunks with the decoder weights,
            # ordered by first-use time (the DMA FIFO is strictly serial)
            w1pa = wp.tile([D, D, 128], F32)
            w1pb = wp.tile([D, D, 128], F32)
            W2sb = wp.tile([128, D, 2, DF], F32)

            def w2_load(i):
                nc.sync.dma_start(
                    out=r(W2sb[:, i, :, :]),
                    in_=r(W2_d[i].rearrange("(k p) c -> p k c", k=2)))

            def eps_load(c):
                cs = ts(c, BL // 4)
                nc.sync.dma_start(out=epsT[:, cs], in_=epsT_d[:, cs])

            def xor_load(c):
                cs = ts(c, BL // 4)
                nc.sync.dma_start(out=r(xorT[:, cs]), in_=r(xorT_d[:, cs]))

            def w1_load(lo, hi):
                nc.sync.dma_start(
                    out=r(w1pa[:, lo:hi, :]),
                    in_=r(W1_d[lo:hi, :, 0:128].rearrange("i k c -> k i c")))
                nc.sync.dma_start(
                    out=r(w1pb[:, lo:hi, :]),
                    in_=r(W1_d[lo:hi, :, 128:256].rearrange("i k c -> k i c")))

            eps_load(0)
            xor_load(1)
            eps_load(1)
            # FIFO ordered by first-use time. Only nodes 0-1 of w1p are
            # touched inside the encoder-merge window (node i first reads at
            # dec step 8i), so just 65KB of L1 weights ride ahead of w2(0) --
            # which gates l2(0) at ~11us -- and xor2/eps2 aren't stuck
            # behind w2(1) (first read ~27us, streamed from the loop).
            w1_load(0, 2)
            w2_load(0)
            xor_load(2)
            eps_load(2)
            xor_load(3)
            eps_load(3)
            # w1p nodes 2:32 and W2 nodes 1+ stream from inside the loop

            with (
                tc.tile_pool(name="hid1", bufs=int(os.environ.get("VAE_H1B", "4"))) as h1p,
                tc.tile_pool(name="hid2", bufs=3) as h2p,
                tc.tile_pool(name="smallp", bufs=2) as smp,
                tc.tile_pool(name="tmp1", bufs=2) as tmp1,
            ):
                # ---- encoder, feature-major, 1-step software pipeline ----
                enc_st = {}

                def enc_mm(bt):
                    bs = ts(bt, BT)
                    p1a = psp.tile([128, BT], F32, tag="ps", name="p1a")
                    nc.tensor.matmul(p1a[:], r(We1sb[:, 0:128]), r(xorT[:, bs]),
                                     start=True, stop=True)
                    p1b = psp.tile([128, BT], F32, tag="ps", name="p1b")
                    nc.tensor.matmul(p1b[:], r(We1sb[:, 128:256]), r(xorT[:, bs]),
                                     start=True, stop=True)
                    enc_st[bt] = (p1a, p1b)

                def enc_relu(bt):
                    p1a, p1b = enc_st.pop(bt)
                    h1a = h1p.tile([128, BT], F32, tag="t1a", name="h1a")
                    nc.scalar.activation(r(h1a[:]), p1a[:], AF.Relu, bias=be1sb[:, 0:1])
                    h1b = h1p.tile([128, BT], F32, tag="t1b", name="h1b")
                    if os.environ.get("VAE_EH1B", "dve") == "pool":
                        # h1b's consumer is NEXT step's p2 matmul (~1.7us of
                        # slack), so Pool's launch latency is survivable here
                        # -- unlike h2a/h2b which feed pz in the same step
                        nc.gpsimd.tensor_scalar(r(h1b[:]), p1b[:],
                                                be1sb[:, 1:2], 0.0,
                                                AluOpType.add, AluOpType.max)
                    else:
                        nc.vector.tensor_scalar(r(h1b[:]), p1b[:],
                                                be1sb[:, 1:2], 0.0,
                                                AluOpType.add, AluOpType.max)
                    enc_st[bt] = (h1a, h1b)

                pz_pair = {}

                def enc_back(bt):
                    bs = ts(bt, BT)
                    h1a, h1b = enc_st.pop(bt)
                    p2a = psp.tile([128, BT], F32, tag="ps", name="p2a")
                    nc.tensor.matmul(p2a[:], r(We2sb[:, 0, 0:128]), r(h1a[:]),
                                     start=True, stop=False)
                    nc.tensor.matmul(p2a[:], r(We2sb[:, 1, 0:128]), r(h1b[:]),
                                     start=False, stop=True)
                    p2b = psp.tile([128, BT], F32, tag="ps", name="p2b")
                    nc.tensor.matmul(p2b[:], r(We2sb[:, 0, 128:256]), r(h1a[:]),
                                     start=True, stop=False)
                    nc.tensor.matmul(p2b[:], r(We2sb[:, 1, 128:256]), r(h1b[:]),
                                     start=False, stop=True)
                    h2a = h2p.tile([128, BT], F32, tag="t2a", name="h2a")
                    nc.scalar.activation(r(h2a[:]), p2a[:], AF.Relu, bias=be2sb[:, 0:1])
                    # h2b also on Act by default: the encoder reparam chain
                    # (below) loads DVE, Act has the headroom
                    h2b = h2p.tile([128, BT], F32, tag="t2b", name="h2b")
                    h2b_eng = os.environ.get("VAE_H2B", "act")
                    h2bsp = int(os.environ.get("VAE_H2BSPLIT", "0"))
                    if h2bsp > 0:
                        # column-split across Act/DVE: the encoder cadence is
                        # the per-tile max of the two engines' serial chains,
                        # and splitting the 4th Act op rebalances them
                        nc.scalar.activation(r(h2b[:, 0:h2bsp]),
                                             p2b[:, 0:h2bsp], AF.Relu,
                                             bias=be2sb[:, 1:2])
                        nc.vector.tensor_scalar(r(h2b[:, h2bsp:BT]),
                                                p2b[:, h2bsp:BT],
                                                be2sb[:, 1:2], 0.0,
                                                AluOpType.add, AluOpType.max)
                    elif h2b_eng == "act":
                        nc.scalar.activation(r(h2b[:]), p2b[:], AF.Relu,
                                             bias=be2sb[:, 1:2])
                    elif h2b_eng == "pool":
                        # Pool is idle during the encoder; Act/DVE are the
                        # encoder pace-setters
                        nc.gpsimd.tensor_scalar(r(h2b[:]), p2b[:],
                                                be2sb[:, 1:2], 0.0,
                                                AluOpType.add, AluOpType.max)
                    else:
                        nc.vector.tensor_scalar(r(h2b[:]), p2b[:],
                                                be2sb[:, 1:2], 0.0,
                                                AluOpType.add, AluOpType.max)
                    if PZPAIR:
                        j, half = divmod(bt, 2)
                        if half == 0:
                            pz_pair[j] = psz.tile([2 * D, 2 * BT], F32,
                                                  tag="pz2", name="pz2")
                        pz2 = pz_pair[j]
                        pzv = pz2[:, half * BT:(half + 1) * BT]
                        nc.tensor.matmul(pzv, r(We3sb[:, 0, :]), r(h2a[:]),
                                         start=True, stop=False)
                        nc.tensor.matmul(pzv, r(We3sb[:, 1, :]), r(h2b[:]),
                                         start=False, stop=True)
                        if half == 0:
                            return
                        # pair complete: one 1024-wide reparam tail
                        pz = pz_pair.pop(j)
                        bs = slice((bt - 1) * BT, (bt + 1) * BT)
                        wid = 2 * BT
                    else:
                        pz = psp.tile([2 * D, BT], F32, tag="ps", name="pz")
                        nc.tensor.matmul(pz[:], r(We3sb[:, 0, :]), r(h2a[:]),
                                         start=True, stop=False)
                        nc.tensor.matmul(pz[:], r(We3sb[:, 1, :]), r(h2b[:]),
                                         start=False, stop=True)
                        wid = BT
                    # et = exp((lv_raw+be3_lv)/2) straight from psum (be3x lv
                    # rows are pre-halved); y0 = (mu_raw + be3_mu) + et*eps
                    nc.scalar.activation(etT[:, bs], pz[D:2 * D, :], AF.Exp,
                                         scale=0.5, bias=be3sb[D:2 * D, :])
                    tm = tmp1.tile([D, wid], F32, tag="tm", name="tm")
                    if os.environ.get("VAE_TM", "dve") == "pool":
                        nc.gpsimd.tensor_mul(tm[:], etT[:, bs], epsT[:, bs])
                    else:
                        nc.vector.tensor_mul(tm[:], etT[:, bs], epsT[:, bs])
                    nc.vector.scalar_tensor_tensor(
                        r(yT[:, bs]), pz[0:D, :], be3sb[0:D, :], tm[:],
                        AluOpType.add, AluOpType.add)

                # (the encoder loop is below, merged with the decoder start)

                # ---- sequential causal decoder ----
                # Software-pipelined emission: the in-order engine queues
                # preserve emission order, so interleaving stages of
                # consecutive iterations is what lets iteration k+1's L1 run
                # while k's relu/L2 are still in flight.
                NIT = D * NBT
                st1, st2, st3, st4 = {}, {}, {}, {}

                def stage_l1(k):
                    i, b = divmod(k, NBT)
                    ke = i + 1
                    bs = ts(b, BT)
                    p1a = psp.tile([128, BT], F32, tag="ps", name="p1a")
                    nc.tensor.matmul(p1a[:], r(w1pa[0:ke, i, :]),
                                     r(yT[0:ke, bs]), start=True, stop=True)
                    p1b = psp.tile([128, BT], F32, tag="ps", name="p1b")
                    nc.tensor.matmul(p1b[:], r(w1pb[0:ke, i, :]),
                                     r(yT[0:ke, bs]), start=True, stop=True)
                    st1[k] = (i, b, p1a, p1b)

                def stage_relu1(k):
                    i, b, p1a, p1b = st1.pop(k)
                    t1a = h1p.tile([128, BT], F32, tag="t1a", name="t1a")
                    if k < KPOOL:
                        nc.gpsimd.tensor_scalar(r(t1a[:]), p1a[:],
                                                b1sb[:, i, 0:1], 0.0,
                                                AluOpType.add, AluOpType.max)
                    else:
                        nc.scalar.activation(r(t1a[:]), p1a[:], AF.Relu,
                                             bias=b1sb[:, i, 0:1])
                    t1b = h1p.tile([128, BT], F32, tag="t1b", name="t1b")
                    nc.vector.tensor_scalar(r(t1b[:]), p1b[:], b1sb[:, i, 1:2],
                                            0.0, AluOpType.add, AluOpType.max)
                    st2[k] = (i, b, t1a, t1b)

                def stage_l2(k):
                    i, b, t1a, t1b = st2.pop(k)
                    p2a = psp.tile([128, BT], F32, tag="ps", name="p2a")
                    nc.tensor.matmul(p2a[:], r(W2sb[:, i, 0, 0:128]), r(t1a[:]),
                                     start=True, stop=False)
                    nc.tensor.matmul(p2a[:], r(W2sb[:, i, 1, 0:128]), r(t1b[:]),
                                     start=False, stop=True)
                    p2b = psp.tile([128, BT], F32, tag="ps", name="p2b")
                    nc.tensor.matmul(p2b[:], r(W2sb[:, i, 0, 128:256]), r(t1a[:]),
                                     start=True, stop=False)
                    nc.tensor.matmul(p2b[:], r(W2sb[:, i, 1, 128:256]), r(t1b[:]),
                                     start=False, stop=True)
                    st3[k] = (i, b, p2a, p2b)

                def stage_relu2(k):
                    i, b, p2a, p2b = st3.pop(k)
                    t2a = h2p.tile([128, BT], F32, tag="t2a", name="t2a")
                    nc.scalar.activation(r(t2a[:]), p2a[:], AF.Relu,
                                         bias=b2sb[:, i, 0:1])
                    t2b = h2p.tile([128, BT], F32, tag="t2b", name="t2b")
                    nc.vector.tensor_scalar(r(t2b[:]), p2b[:], b2sb[:, i, 1:2],
                                            0.0, AluOpType.add, AluOpType.max)
                    st4[k] = (i, b, t2a, t2b)

                def stage_l3(k):
                    i, b, t2a, t2b = st4.pop(k)
                    bs = ts(b, BT)
                    p3 = ps3.tile([1, BT], F32, tag="p3", name="p3")
                    nc.tensor.matmul(p3[:], r(W3sb[:, i, 0:1]), r(t2a[:]),
                                     start=True, stop=False)
                    nc.tensor.matmul(p3[:], r(W3sb[:, i, 1:2]), r(t2b[:]),
                                     start=False, stop=True)
                    # the decoded row must cross partitions (psum partition 0
                    # -> yT partition i), which only DMA can do: stage
                    # through SBUF then one row DMA. ~60% of the copies on
                    # Act, 40% on DVE: balances the two (DVE carries the
                    # relu-b chain + encoder reparam; Act carries relu-a+et)
                    srow = smp.tile([1, BT], F32, tag="srow", name="srow",
                                    bufs=3)
                    if k % SROW_MOD >= SROW_THR:
                        nc.vector.tensor_copy(srow[:], p3[:])
                    else:
                        nc.scalar.activation(srow[:], p3[:], AF.Copy)
                    if i < D - 1:
                        nc.sync.dma_start(out=r(yT[i:i + 1, bs]),
                                          in_=r(srow[:]))
                    else:
                        # node 31 feeds nothing downstream: stream the row
                        # straight to the output
                        nc.sync.dma_start(out=yT_dr[D - 1:D, bs], in_=srow[:])

                def yfix_rows(b):
                    # rows 0..30 of tile b are final once node 30's row DMA
                    # lands: stream them out, overlapping node-31 compute
                    # (b3 is added back on the host at gather time)
                    nc.sync.dma_start(out=yT_dr[0:D - 1, ts(b, BT)],
                                      in_=yT[0:D - 1, ts(b, BT)])

                # 4-deep pipeline: every PE stage consumes only results from
                # strictly earlier steps, so PE never waits on same-step
                # vector work.
                def dec_step(k):
                    if k < NIT:
                        stage_l1(k)
                        stage_relu1(k)
                    if 2 <= k < NIT + 2:
                        stage_relu2(k - 2)
                    if 3 <= k:
                        stage_l3(k - 3)
                    if 1 <= k < NIT + 1:
                        stage_l2(k - 1)
                    dec_dmas(k)

                def dec_dmas(k):
                    # widely-spaced bulk DMAs, between the row DMAs they
                    # must not delay. The y0/et outputs for tile b go at step
                    # b: the DMA FIFO then guarantees the y0 read of yT[b]
                    # row 0 completes before node 0's row DMA (step b+3)
                    # overwrites it.
                    # y0 out in PAIRS (one descriptor per two tiles: the DMA
                    # FIFO costs ~1.3us per descriptor, flat). Pair (b, b+1)
                    # goes at step b+1: tile b+1's reparam STT has landed by
                    # then, and the FIFO still orders it ahead of node-0's
                    # row DMA for tile b (step b+3) which overwrites row 0.
                    if k < NBT:
                        bs = ts(k, BT)
                        nc.sync.dma_start(out=y0T_d[:, bs], in_=yT[:, bs])
                    # etT is persistent after the encoder: stream it out in
                    # the mid-decode lull instead of the congested start,
                    # also in pairs
                    etk = int(os.environ.get("VAE_ETK", "160"))
                    if etk <= k < etk + 2 * NBT and (k - etk) % 2 == 0:
                        b = (k - etk) // 2
                        nc.sync.dma_start(out=etT_d[:, ts(b, BT)],
                                          in_=etT[:, ts(b, BT)])
                    if k == int(os.environ.get("VAE_W2_1K", "2")):
                        w2_load(1)
                    if k == int(os.environ.get("VAE_W1K", "3")):
                        w1_load(2, 8)
                    if k == NBT + 1:
                        w2_load(2)
                    if k == NBT + 3:
                        w2_load(3)
                    # w1p nodes 8:32 in 8-node chunks spread over the early
                    # decode (node i's weights needed at step 8i; chunk c
                    # covers nodes 8c+8..8c+16, needed from step 64(c+1))
                    w1spread = os.environ.get("VAE_W1SPREAD", "1") == "1"
                    if w1spread and k in (20, 28, 36):
                        c = (k - 20) // 8
                        nc.sync.dma_start(
                            out=r(w1pa[:, 8 * c + 8:8 * c + 16, :]),
                            in_=r(W1_d[8 * c + 8:8 * c + 16, :, 0:128]
                                  .rearrange("i k c -> k i c")))
                    if w1spread and k in (24, 32, 40):
                        c = (k - 24) // 8
                        nc.sync.dma_start(
                            out=r(w1pb[:, 8 * c + 8:8 * c + 16, :]),
                            in_=r(W1_d[8 * c + 8:8 * c + 16, :, 128:256]
                                  .rearrange("i k c -> k i c")))
                    if not w1spread and k == 2 * NBT:
                        nc.sync.dma_start(
                            out=r(w1pa[:, 8:D, :]),
                            in_=r(W1_d[8:D, :, 0:128].rearrange("i k c -> k i c")))
                    if not w1spread and k == 2 * NBT + 2:
                        nc.sync.dma_start(
                            out=r(w1pb[:, 8:D, :]),
                            in_=r(W1_d[8:D, :, 128:256].rearrange("i k c -> k i c")))
                    if k < NIT and k % NBT == 5 and 4 <= k // NBT + 4 < D:
                        w2_load(k // NBT + 4)
                    if (D - 1) * NBT <= k < D * NBT:
                        yfix_rows(k - (D - 1) * NBT)

                # encoder loop, with the first decoder steps interleaved from
                # s=MERGE0 on: the encoder is DVE/Act-paced (reparam chain),
                # so PE has idle slots the early decoder matmuls can fill.
                # Per-step order: next tile's L1 matmuls first (keeps PE fed
                # without waiting on fresh relus), then (DECFIRST) the merged
                # decoder L1 -- its matmuls depend only on state from 8 steps
                # back, so they keep PE busy while Act/DVE catch up on the
                # relus that enc_back's pz matmuls block on -- then the
                # previous tile's back half, then this tile's relus.
                decfirst = os.environ.get("VAE_DECFIRST", "0")

                def dec_front(k):
                    if k < NIT:
                        stage_l1(k)
                        if decfirst == "1":
                            stage_relu1(k)

                def dec_rest(k):
                    if decfirst == "2" and k < NIT:
                        stage_relu1(k)
                    if 2 <= k < NIT + 2:
                        stage_relu2(k - 2)
                    if 3 <= k:
                        stage_l3(k - 3)
                    if 1 <= k < NIT + 1:
                        stage_l2(k - 1)
                    dec_dmas(k)

                # from s=M2 on, two decoder steps per encoder step: the
                # encoder window is Act-bound, so every extra merged step
                # swaps PE-idle time for work the PE-bound tail won't redo
                M2 = int(os.environ.get("VAE_M2", "99"))
                nextk = 0
                for s in range(NBT + 1):
                    if s < NBT:
                        enc_mm(s)
                    if decfirst != "0" and s >= MERGE0:
                        dec_front(nextk)
                    if s >= 1:
                        enc_back(s - 1)
                    if s < NBT:
                        enc_relu(s)
                    if s >= MERGE0:
                        if decfirst != "0":
                            dec_rest(nextk)
                            nextk += 1
                        else:
                            dec_step(nextk)
                            nextk += 1
                        if s >= M2:
                            dec_step(nextk)
                            nextk += 1
                for k in range(nextk, NIT + 3):
                    dec_step(k)

    nc.compile()
    return nc


_NC_CACHE = None


def _get_nc():
    global _NC_CACHE
    if _NC_CACHE is None:
        _NC_CACHE = _make_nc()
    return _NC_CACHE


def _pack_pdm(a):
    """[D, 256] -> [128, D, 2] with out[p, i, m] = a[i, m*128+p]."""
    return np.ascontiguousarray(
        a.reshape(D, 2, 128).transpose(2, 0, 1), dtype=np.float32)


def _run(inputs, trace=False):
    f32c = lambda a: np.ascontiguousarray(np.asarray(a), dtype=np.float32)
    xorT = f32c(inputs["xor"]).T   # [D, B]
    epsT = f32c(inputs["eps"]).T
    shared = {k: f32c(inputs[k]) for k in ["We1", "W1", "W2"]}
    # Decoded y rows live on-device WITHOUT b3 (pure psum row DMAs);
    # node i's L1 input is then short by sum_{k<i} b3[k]*W1[i][k,:], a
    # weight-only constant folded into the relu1 bias here. b3 is added
    # back at output time. Small weights are pre-packed into their
    # on-chip layouts AND bundled so the startup FIFO (flat ~1us per DMA)
    # carries 3 descriptors instead of 8.
    W1 = f32c(inputs["W1"]).astype(np.float64)
    b3 = f32c(inputs["b3"]).astype(np.float64)
    mask = np.tril(np.ones((D, D)), -1)
    corr = np.einsum("ik,k,ikc->ic", mask, b3, W1)
    b1mod = (f32c(inputs["b1"]).astype(np.float64) + corr).astype(np.float32)
    # bep = [be1 | be2] as [128, 4]
    shared["bep"] = np.ascontiguousarray(np.concatenate(
        [f32c(inputs["be1"]).reshape(2, 128).T,
         f32c(inputs["be2"]).reshape(2, 128).T], axis=1))
    # Wep = [We2 | We3] in the on-chip [128, 2, DF+2D] layout
    w2e = f32c(inputs["We2"]).reshape(2, 128, DF).transpose(1, 0, 2)
    w3e = f32c(inputs["We3"]).reshape(2, 128, 2 * D).transpose(1, 0, 2)
    shared["Wep"] = np.ascontiguousarray(
        np.concatenate([w2e, w3e], axis=2))
    # bwp = [b1p | b2p | W3p | be3pad] as [128, 6D+1]; be3 packed as
    # [be3_mu ; be3_lv/2] (lv half pre-halved for the fused
    # exp((lv+be3_lv)/2) activation), zero-padded to 128 rows
    be3 = f32c(inputs["be3"])
    be3x = np.concatenate([be3[0:D], be3[D:2 * D] / 2.0])
    be3col = np.zeros((128, 1), np.float32)
    be3col[0:2 * D, 0] = be3x
    shared["bwp"] = np.ascontiguousarray(np.concatenate(
        [_pack_pdm(b1mod).reshape(128, 2 * D),
         _pack_pdm(f32c(inputs["b2"])).reshape(128, 2 * D),
         _pack_pdm(f32c(inputs["W3"])).reshape(128, 2 * D),
         be3col], axis=1))
    in_maps = []
    for c in range(NCORES):
        m = dict(shared)
        m["xorT"] = np.ascontiguousarray(xorT[:, c * BL:(c + 1) * BL])
        m["epsT"] = np.ascontiguousarray(epsT[:, c * BL:(c + 1) * BL])
        in_maps.append(m)
    nc = _get_nc()
    res = run_bass_kernel_spmd(nc, in_maps, core_ids=list(range(NCORES)),
                               trace=trace)
    gather = lambda nm: np.ascontiguousarray(
        np.concatenate([r[nm] for r in res.results], axis=1).T)
    # decoded rows come back without b3 (folded out on-device); add it here.
    # mu/lv are reconstructed exactly from the device's et=exp(lv/2) and
    # y0 = mu + et*eps streams.
    y = gather("yT") + f32c(inputs["b3"])[None, :]
    et = gather("etT").astype(np.float64)
    y0 = gather("y0T").astype(np.float64)
    lv = (2.0 * np.log(et)).astype(np.float32)
    mu = (y0 - et * f32c(inputs["eps"]).astype(np.float64)).astype(np.float32)
    return (y, mu, lv, y), res


def kernel(**inputs):
    out, _ = _run(inputs)
    return out

